# revision 1
# baseline (speedup 1.0000x reference)
"""Trainium2 Bass kernel for 3-layer GCN + Linear + log_softmax.

nn_GCN3_Lin1: x[100000,16], edge_index[2,6400000] ->
  h = relu(gcn(x;W1,b1)); h = relu(gcn(h;W2,b2)); h = relu(gcn(h;W3,b3))
  out = log_softmax(h @ Wf + bf)

Strategy (8 NeuronCores, graph/data parallel over nodes):
  - Nodes are degree-sorted and dealt round-robin to 8 cores; each core owns
    `slots = n_groups*128` node slots (tail slots are zero dummies).
  - GCN normalization is factored: with u = dinv*h (dinv = rsqrt(1+indeg)),
    out[d] = dinv[d]*(sum_{e:dst=d} u[src_e] + u[d]) + b.  No per-edge float
    work beyond the row sums; the self-loop term is the local row.
  - Per layer: per-128-node-group GEMM on PE -> dinv scale -> shard write ->
    AllGather into a full node-major table in DRAM -> per-group indirect-DMA
    gather of all in-edges (host-built padded index lists; pads point at an
    always-zero row) -> strided tensor_reduce over the padded slot axis ->
    scale/bias/relu -> PE transpose to feed the next GEMM.
  - In-degrees are computed on device from the pad pattern of the index
    lists; host-side preprocessing is pure integer index manipulation.
"""

import math

import numpy as np

from concourse import bass, mybir, bacc, tile
from concourse.bass_utils import run_bass_kernel_spmd
from concourse.masks import make_identity

F32 = mybir.dt.float32
I32 = mybir.dt.int32
GROUP = 128
N_CORES = 8
DIMS = (16, 32, 24, 12, 6)

LAST_RUN_INFO = {}


# ---------------------------------------------------------------------------
# Host-side plan (pure index manipulation)
# ---------------------------------------------------------------------------
class _Plan:
    pass


def _build_plan(edge_index, n_nodes, n_cores=N_CORES):
    src = np.asarray(edge_index[0], dtype=np.int64)
    dst = np.asarray(edge_index[1], dtype=np.int64)
    assert n_nodes % n_cores == 0
    per_core = n_nodes // n_cores
    n_groups = math.ceil(per_core / GROUP)
    slots = n_groups * GROUP
    assert per_core < slots, "need at least one dummy slot for the zero/pad row"

    deg_in = np.bincount(dst, minlength=n_nodes).astype(np.int64)

    order = np.argsort(-deg_in, kind="stable")
    ranks = np.empty(n_nodes, dtype=np.int64)
    ranks[order] = np.arange(n_nodes)
    core_of = ranks % n_cores
    q_of = ranks // n_cores
    g_of = q_of // GROUP
    p_of = q_of % GROUP
    slot_of = p_of * n_groups + g_of
    table_row = core_of * slots + slot_of

    Pg = np.zeros(n_groups, dtype=np.int64)
    np.maximum.at(Pg, g_of, deg_in)
    Pg = np.maximum(2, ((Pg + 1) // 2) * 2).astype(np.int64)
    off = np.zeros(n_groups + 1, dtype=np.int64)
    off[1:] = np.cumsum(Pg)
    S_tot = int(off[-1])

    PAD_ROW = slots - 1

    # int32 padded lists (p-major slots) — used only for on-device degree calc
    idx_all = np.full((n_cores, GROUP, S_tot), PAD_ROW, dtype=np.int32)
    dst_core = core_of[dst]
    dst_q = q_of[dst]
    src_row = table_row[src].astype(np.int32)
    for c in range(n_cores):
        m = dst_core == c
        q_c = dst_q[m]
        s_c = src_row[m]
        o = np.argsort(q_c, kind="stable")
        q_s = q_c[o]
        s_s = s_c[o]
        node_start = np.searchsorted(q_s, np.arange(per_core))
        j = np.arange(len(q_s)) - node_start[q_s]
        g = q_s // GROUP
        p = q_s % GROUP
        idx_all[c][p, off[g] + j] = s_s

    # ---- int16 call plan for dma_gather (Ant ucode) ----
    # sub-tables of `cps` cores each so local rows fit int16; zero/pad row of
    # sub-table b is its first core's dummy slot (local row slots-1)
    cps = max(1, 32768 // slots)
    cps = min(cps, n_cores)
    n_sub = math.ceil(n_cores / cps)
    sub_rows = cps * slots
    assert sub_rows - (cps - 1) * slots - 1 <= 32767

    src_core = core_of[src]
    src_sub = src_core // cps
    src_local = table_row[src] - src_sub * sub_rows

    # per (group, sub) padded slot count, max over cores
    cnt_gs = np.zeros((n_cores, per_core, n_sub), dtype=np.int32)
    np.add.at(cnt_gs, (dst_core, dst_q, src_sub), 1)
    group_of_q = np.repeat(np.arange(n_groups), GROUP)[:per_core]
    Pgs = np.zeros((n_groups, n_sub), dtype=np.int64)
    for b in range(n_sub):
        tmp = np.zeros(n_groups, dtype=np.int64)
        np.maximum.at(tmp, group_of_q, cnt_gs[:, :, b].max(axis=0))
        Pgs[:, b] = tmp
    for g in range(n_groups):
        if Pgs[g].sum() == 0:
            Pgs[g, 0] = 1  # all-pad call keeps the reduce well-defined

    # calls: (g, b, cnt) chunks of <=64 slots (num_idxs <= 8192 per ucode call)
    CH = 64
    calls = []  # (g, b, slot_off_in_group_tile, cnt, col16_off)
    slot_tot = np.zeros(n_groups, dtype=np.int64)
    col16 = 0
    for g in range(n_groups):
        so = 0
        for b in range(n_sub):
            s0 = 0
            while s0 < int(Pgs[g, b]):
                cnt = min(CH, int(Pgs[g, b]) - s0)
                calls.append((g, b, so, cnt, col16))
                col16 += cnt * 8
                so += cnt
                s0 += cnt
        slot_tot[g] = so
    TOT16 = col16

    # per-core wrapped int16 index arrays (idx i of a call -> [i%16 (+16r), i//16])
    PAD_LOCAL = slots - 1  # sub-table's first core's dummy slot (always zero)
    idx16_all = np.full((n_cores, GROUP, TOT16), PAD_LOCAL, dtype=np.int16)
    base = np.full((n_groups, n_sub, 16), -1, dtype=np.int64)
    for g, b, so, cnt, c16 in calls:
        ci = 0
        while base[g, b, ci] >= 0:
            ci += 1
        base[g, b, ci] = c16
    for c in range(n_cores):
        m = dst_core == c
        q_c = dst_q[m]
        b_c = src_sub[m]
        l_c = src_local[m]
        o = np.argsort(q_c * n_sub + b_c, kind="stable")
        q_s = q_c[o]
        b_s = b_c[o]
        l_s = l_c[o].astype(np.int16)
        key = q_s * n_sub + b_s
        starts = np.searchsorted(key, np.arange(per_core * n_sub))
        j = np.arange(len(key)) - starts[key]
        g_s = q_s // GROUP
        p_s = q_s % GROUP
        chunk = j // CH
        s_in = j % CH
        c16_e = base[g_s, b_s, chunk]
        assert (c16_e >= 0).all()
        i_flat = s_in * GROUP + p_s
        col = c16_e + i_flat // 16
        prow = i_flat % 16
        for rep in range(8):
            idx16_all[c][prow + rep * 16, col] = l_s

    pl = _Plan()
    pl.n_cores = n_cores
    pl.n_nodes = n_nodes
    pl.per_core = per_core
    pl.n_groups = n_groups
    pl.slots = slots
    pl.Pg = Pg
    pl.off = off
    pl.S_tot = S_tot
    pl.PAD_ROW = PAD_ROW
    pl.idx_all = idx_all
    pl.cps = cps
    pl.n_sub = n_sub
    pl.sub_rows = sub_rows
    pl.calls = calls
    pl.slot_tot = slot_tot
    pl.TOT16 = TOT16
    pl.idx16_all = idx16_all
    pl.core_of = core_of
    pl.q_of = q_of
    pl.g_of = g_of
    pl.p_of = p_of
    return pl


def _make_in_maps(pl, x, W1, b1, W2, b2, W3, b3, Wf, bf):
    d_in = x.shape[1]
    in_maps = []
    for c in range(pl.n_cores):
        own = pl.core_of == c
        xT = np.zeros((d_in, pl.slots), dtype=np.float32)
        xT[:, pl.q_of[own]] = x[own].T
        in_maps.append(
            {
                "xT_in": xT,
                "idx_in": np.ascontiguousarray(pl.idx_all[c]),
                "idx16_in": np.ascontiguousarray(pl.idx16_all[c]),
                "W1": np.asarray(W1, np.float32),
                "W2": np.asarray(W2, np.float32),
                "W3": np.asarray(W3, np.float32),
                "Wf": np.asarray(Wf, np.float32),
                "b1": np.tile(np.asarray(b1, np.float32)[None, :], (GROUP, 1)),
                "b2": np.tile(np.asarray(b2, np.float32)[None, :], (GROUP, 1)),
                "b3": np.tile(np.asarray(b3, np.float32)[None, :], (GROUP, 1)),
                "bf": np.tile(np.asarray(bf, np.float32)[None, :], (GROUP, 1)),
            }
        )
    return in_maps


def _assemble_output(pl, outs_per_core, d_out):
    full = np.empty((pl.n_nodes, d_out), dtype=np.float32)
    rows = pl.p_of * pl.n_groups + pl.g_of
    for c in range(pl.n_cores):
        own = pl.core_of == c
        full[own] = outs_per_core[c][rows[own]]
    return full


# ---------------------------------------------------------------------------
# Device kernel
# ---------------------------------------------------------------------------
def _build_kernel(pl, dims=DIMS, gather_bufs=3, debug=False):
    d0, d1, d2, d3, d4 = dims
    NG, S, SL = pl.n_groups, pl.S_tot, pl.slots
    NC = pl.n_cores
    Pg, off = pl.Pg, pl.off
    dmax = max(d1, d2, d3)

    nc = bacc.Bacc("TRN2", target_bir_lowering=False, debug=False, num_devices=NC)

    xT_in = nc.dram_tensor("xT_in", [d0, SL], F32, kind="ExternalInput")
    idx_in = nc.dram_tensor("idx_in", [GROUP, S], I32, kind="ExternalInput")
    idx16_in = nc.dram_tensor(
        "idx16_in", [GROUP, pl.TOT16], mybir.dt.int16, kind="ExternalInput"
    )
    wdims = {"W1": (d0, d1), "W2": (d1, d2), "W3": (d2, d3), "Wf": (d3, d4)}
    bdims = {"b1": d1, "b2": d2, "b3": d3, "bf": d4}
    Ws = {
        n: nc.dram_tensor(n, list(ab), F32, kind="ExternalInput")
        for n, ab in wdims.items()
    }
    bs = {
        n: nc.dram_tensor(n, [GROUP, d], F32, kind="ExternalInput")
        for n, d in bdims.items()
    }
    out_dram = nc.dram_tensor("out", [SL, d4], F32, kind="ExternalOutput")
    if debug:
        dbg_dinv = nc.dram_tensor("dbg_dinv", [GROUP, NG], F32, kind="ExternalOutput")
        dbg_hown1 = nc.dram_tensor("dbg_hown1", [GROUP, NG * dims[1]], F32, kind="ExternalOutput")
        dbg_tab = nc.dram_tensor("dbg_tab", [2 * SL, 64], F32, kind="ExternalOutput")
        dbg_z1 = nc.dram_tensor("dbg_z1", [GROUP, NG * dims[1]], F32, kind="ExternalOutput")

    EL = 64  # table row width (256B rows for the Ant gather ucode)
    shard = {}
    table = {}
    for k in (1, 2, 3):
        shard[k] = nc.dram_tensor(f"shard{k}", [SL, EL], F32)
        table[k] = nc.dram_tensor(f"table{k}", [NC * SL, EL], F32, addr_space="Shared")

    rgroups = [list(range(NC))]

    with tile.TileContext(nc, num_cores=NC) as tc:
        with (
            tc.tile_pool(name="persist", bufs=1) as pers,
            tc.tile_pool(name="gat", bufs=gather_bufs) as gpool,
            tc.tile_pool(name="idx16", bufs=6) as ipool,
            tc.tile_pool(name="work", bufs=4) as wpool,
            tc.tile_pool(name="ps", bufs=4, space="PSUM") as ppool,
            tc.tile_pool(name="pst", bufs=4, space="PSUM") as ppool2,
        ):
            idx_sb = gpool.tile([GROUP, S], I32, name="idx_sb", tag="gat")
            nc.sync.dma_start(idx_sb[:], idx_in[:, :])
            # pre-zero the 64-wide shard bounce buffers (pad cols stay zero)
            zsb = gpool.tile([GROUP, SL * EL // GROUP], F32, name="zsb", tag="gat")
            nc.vector.memset(zsb[:], 0.0)
            for k in (1, 2, 3):
                nc.sync.dma_start(
                    shard[k].ap().rearrange("(p r) f -> p (r f)", p=GROUP), zsb[:]
                )
            xT_sb = pers.tile([dmax, SL], F32)
            nc.sync.dma_start(xT_sb[:d0, :], xT_in[:, :])
            W_sb = {}
            for n, ab in wdims.items():
                W_sb[n] = pers.tile(list(ab), F32, name=f"Wsb_{n}")
                nc.sync.dma_start(W_sb[n][:], Ws[n][:, :])
            b_sb = {}
            for n, d in bdims.items():
                b_sb[n] = pers.tile([GROUP, d], F32, name=f"bsb_{n}")
                nc.sync.dma_start(b_sb[n][:], bs[n][:, :])

            ident = pers.tile([GROUP, GROUP], F32)
            make_identity(nc, ident[:])

            # ---- in-degree -> dinv [128, NG] (from pad pattern) ----
            idx_f = gpool.tile([GROUP, S], F32, name="idx_f", tag="gat")
            nc.vector.tensor_copy(idx_f[:], idx_sb[:])
            ispad = gpool.tile([GROUP, S], F32, name="ispad", tag="gat")
            nc.vector.tensor_scalar(
                out=ispad[:],
                in0=idx_f[:],
                scalar1=float(pl.PAD_ROW),
                scalar2=None,
                op0=mybir.AluOpType.is_equal,
            )
            deg = pers.tile([GROUP, NG], F32)
            padcnt = pers.tile([GROUP, NG], F32)
            for g in range(NG):
                nc.vector.tensor_reduce(
                    out=padcnt[:, g : g + 1],
                    in_=ispad[:, off[g] : off[g + 1]],
                    axis=mybir.AxisListType.X,
                    op=mybir.AluOpType.add,
                )
                nc.vector.tensor_scalar(
                    out=deg[:, g : g + 1],
                    in0=padcnt[:, g : g + 1],
                    scalar1=-1.0,
                    scalar2=float(Pg[g] + 1),
                    op0=mybir.AluOpType.mult,
                    op1=mybir.AluOpType.add,
                )
            dinv = pers.tile([GROUP, NG], F32)
            nc.vector.reciprocal(dinv[:], deg[:])
            nc.scalar.activation(
                out=dinv[:], in_=dinv[:], func=mybir.ActivationFunctionType.Sqrt
            )

            hown = pers.tile([GROUP, NG * dmax], F32)
            out_sb = pers.tile([GROUP, NG * d4], F32)
            n_dummy_p = SL - pl.per_core

            for k, din, dout, wname, bname in [
                (1, d0, d1, "W1", "b1"),
                (2, d1, d2, "W2", "b2"),
                (3, d2, d3, "W3", "b3"),
            ]:
                # h'own = dinv * (x @ W)
                for g in range(NG):
                    ps = ppool.tile([GROUP, dout], F32, space="PSUM", name=f"ps_{k}_{g}", tag="ps")
                    nc.tensor.matmul(
                        ps[:],
                        lhsT=xT_sb[:din, g * GROUP : (g + 1) * GROUP],
                        rhs=W_sb[wname][:],
                        start=True,
                        stop=True,
                    )
                    nc.vector.tensor_scalar_mul(
                        hown[:, g * dout : (g + 1) * dout], ps[:], dinv[:, g : g + 1]
                    )
                nc.sync.dma_start(
                    shard[k][:, :dout].rearrange("(p g) f -> p g f", g=NG),
                    hown[:, : NG * dout],
                )
                nc.gpsimd.collective_compute(
                    "AllGather",
                    mybir.AluOpType.bypass,
                    replica_groups=rgroups,
                    ins=[shard[k].ap().opt()],
                    outs=[table[k].ap().opt()],
                )
                if debug and k == 1:
                    nc.sync.dma_start(dbg_dinv[:, :], dinv[:])
                    nc.sync.dma_start(dbg_hown1[:, :], hown[:, : NG * dout])
                    tcp = wpool.tile(
                        [GROUP, 2 * SL * 64 // GROUP], F32, name="tcp", tag="tcp", bufs=1
                    )
                    nc.sync.dma_start(
                        tcp[:],
                        table[k][0 : 2 * SL, :].rearrange(
                            "(c p g) f -> p c g f", c=2, p=GROUP
                        ),
                    )
                    nc.sync.dma_start(
                        dbg_tab.ap().rearrange("(c p g) f -> p c g f", c=2, p=GROUP),
                        tcp[:],
                    )
                ci = 0
                for g in range(NG):
                    stot = int(pl.slot_tot[g])
                    gat = gpool.tile(
                        [GROUP, stot * EL], F32, name=f"gat_{k}_{g}", tag="gat"
                    )
                    while ci < len(pl.calls) and pl.calls[ci][0] == g:
                        _, b, so, cnt, c16 = pl.calls[ci]
                        it = ipool.tile(
                            [GROUP, cnt * 8],
                            mybir.dt.int16,
                            name=f"it_{k}_{ci}",
                            tag="it",
                        )
                        nc.sync.dma_start(it[:], idx16_in[:, c16 : c16 + cnt * 8])
                        nc.gpsimd.dma_gather(
                            out_ap=gat[:, so * EL : (so + cnt) * EL].rearrange(
                                "p (s f) -> p s f", f=EL
                            ),
                            in_ap=table[k][b * pl.sub_rows : (b + 1) * pl.sub_rows, :],
                            idxs_ap=it[:],
                            num_idxs=cnt * GROUP,
                            num_idxs_reg=cnt * GROUP,
                            elem_size=EL,
                            single_packet=False,
                        )
                        ci += 1
                    z = wpool.tile([GROUP, dout], F32, name=f"z_{k}_{g}", tag="z")
                    nc.vector.tensor_reduce(
                        out=z[:],
                        in_=gat[:].rearrange("p (s f) -> p f s", f=EL)[:, :dout, :],
                        axis=mybir.AxisListType.X,
                        op=mybir.AluOpType.add,
                    )
                    if debug and k == 1:
                        nc.sync.dma_start(
                            dbg_z1[:, g * dout : (g + 1) * dout], z[:]
                        )
                    nc.vector.tensor_add(z[:], z[:], hown[:, g * dout : (g + 1) * dout])
                    nc.vector.scalar_tensor_tensor(
                        out=z[:],
                        in0=z[:],
                        scalar=dinv[:, g : g + 1],
                        in1=b_sb[bname][:],
                        op0=mybir.AluOpType.mult,
                        op1=mybir.AluOpType.add,
                    )
                    rl = wpool.tile([GROUP, dout], F32, name=f"rl_{k}_{g}", tag="rl")
                    nc.scalar.activation(
                        out=rl[:], in_=z[:], func=mybir.ActivationFunctionType.Relu
                    )
                    pst = ppool2.tile(
                        [dout, GROUP], F32, space="PSUM", name=f"pst_{k}_{g}", tag="pst"
                    )
                    nc.tensor.transpose(out=pst[:], in_=rl[:], identity=ident[:])
                    nc.vector.tensor_copy(
                        xT_sb[:dout, g * GROUP : (g + 1) * GROUP], pst[:]
                    )
                nc.vector.memset(xT_sb[:dout, SL - n_dummy_p : SL], 0.0)

            # ---- final linear + log_softmax ----
            for g in range(NG):
                ps = ppool.tile([GROUP, d4], F32, space="PSUM", name=f"psf_{g}", tag="ps")
                nc.tensor.matmul(
                    ps[:],
                    lhsT=xT_sb[:d3, g * GROUP : (g + 1) * GROUP],
                    rhs=W_sb["Wf"][:],
                    start=True,
                    stop=True,
                )
                logits = wpool.tile([GROUP, d4], F32, name=f"lg_{g}", tag="lg")
                nc.vector.tensor_add(logits[:], ps[:], b_sb["bf"][:])
                m = wpool.tile([GROUP, 1], F32, name=f"m_{g}", tag="m")
                nc.vector.tensor_reduce(
                    out=m[:],
                    in_=logits[:],
                    axis=mybir.AxisListType.X,
                    op=mybir.AluOpType.max,
                )
                negm = wpool.tile([GROUP, 1], F32, name=f"nm_{g}", tag="nm")
                nc.vector.tensor_scalar_mul(negm[:], m[:], -1.0)
                e = wpool.tile([GROUP, d4], F32, name=f"e_{g}", tag="e")
                s = wpool.tile([GROUP, 1], F32, name=f"s_{g}", tag="s")
                nc.scalar.activation(
                    out=e[:],
                    in_=logits[:],
                    func=mybir.ActivationFunctionType.Exp,
                    bias=negm[:],
                    scale=1.0,
                    accum_out=s[:],
                )
                ls = wpool.tile([GROUP, 1], F32, name=f"ls_{g}", tag="ls")
                nc.scalar.activation(
                    out=ls[:], in_=s[:], func=mybir.ActivationFunctionType.Ln
                )
                shift = wpool.tile([GROUP, 1], F32, name=f"sh_{g}", tag="sh")
                nc.vector.tensor_sub(shift[:], negm[:], ls[:])
                nc.vector.tensor_scalar_add(
                    out_sb[:, g * d4 : (g + 1) * d4], logits[:], shift[:]
                )

            nc.sync.dma_start(
                out_dram.ap().rearrange("(p g) f -> p (g f)", g=NG),
                out_sb[:],
            )

    nc.compile()
    return nc


# ---------------------------------------------------------------------------
# Entry point
# ---------------------------------------------------------------------------
def kernel(x, edge_index, W1, b1, W2, b2, W3, b3, Wf, bf):
    x = np.asarray(x, dtype=np.float32)
    n_nodes = x.shape[0]
    pl = _build_plan(np.asarray(edge_index), n_nodes)
    nc = _build_kernel(pl)
    in_maps = _make_in_maps(pl, x, W1, b1, W2, b2, W3, b3, Wf, bf)

    res = run_bass_kernel_spmd(nc, in_maps, core_ids=list(range(pl.n_cores)))

    LAST_RUN_INFO.clear()
    LAST_RUN_INFO["exec_time_ns"] = res.exec_time_ns
    LAST_RUN_INFO["mean_exec_time_ns"] = res.mean_exec_time_ns

    outs = [res.results[c]["out"] for c in range(pl.n_cores)]
    return _assemble_output(pl, outs, d_out=DIMS[-1])



# revision 2
# speedup vs baseline: 1.0120x; 1.0120x over previous
"""Trainium2 Bass kernel for 3-layer GCN + Linear + log_softmax (v2).

The baseline spent its whole 33.8ms in DMAGatherAnt descriptor generation
on the single Q7 (gpsimd) engine (~70 cycles/edge-descriptor).  v2 uses the
`ap_gather` SBUF->SBUF ucode instead: measured 27.1ns per index per core,
with the 8 Q7 cores running 8 independent index streams in parallel and
each index moving a full 32-feature edge message (bf16 d=2 feature-pair
rows, 16 partitions x 2 features) => ~3.4 ns/edge.

Layout: feature-major, nodes sharded 8 ways.  Core-chunk k (partitions
[16k,16k+16)) holds the full u table for shard k: table[16k+p, 2n+a] =
feature (16a+p) of shard-k node n, bf16.  Per layer:

  GEMM (quadrant-split W, K=16 accumulate) on dinv-prescaled activations
  -> u chunks -> DRAM shard -> AllGather -> table (straight DMA, the
  AllGather concat IS the table layout) -> NCALLS ap_gather calls of C
  idxs/core (per-core edge streams bucketed by src shard, dst segments
  rank-sorted by per-chunk in-degree; uniform segment length P per
  (chunk-pair, call)) -> strided tensor_reduce into bf16 partial[rank]
  -> unpermute (ap_gather d=2 over partial) -> masked add of the
  self-loop u_own term -> stacked-identity PE matmuls sum the 8 chunks
  -> z = dinv*S, x' = relu(z+b), prescale by dinv for the next GEMM.

Final layer: logits feature-major; log_softmax = logits - ones^T @
ln(mask6^T @ exp(logits)) via two tiny matmuls.  Host de-shards.
"""

import math

import numpy as np

from concourse import bass, mybir, bacc, tile
from concourse.bass_utils import run_bass_kernel_spmd

F32 = mybir.dt.float32
F16 = mybir.dt.float16
BF16 = mybir.dt.bfloat16
I16 = mybir.dt.int16

N_CORES = 8
PER = 12500
SLOTS = 12544
NP = 4  # chunk pairs
DP = 32
DIMS = (16, 32, 24, 12, 6)
C = 2048  # gather-call indices per core
UCH = 2048  # unpermute chunk columns
ECH = 512  # epilogue / GEMM chunk
PADIDX = SLOTS - 1  # dummy node slot (always-zero table column)

LAST_RUN_INFO = {}


def _bf16(a):
    import ml_dtypes

    return np.asarray(a).astype(ml_dtypes.bfloat16)


def _f16(a):
    return np.asarray(a).astype(np.float16)


class _Plan:
    pass


# ---------------------------------------------------------------------------
# Host-side plan (pure integer index manipulation)
# ---------------------------------------------------------------------------
def _build_plan(edge_index, n_nodes):
    src = np.asarray(edge_index[0], dtype=np.int64)
    dst = np.asarray(edge_index[1], dtype=np.int64)
    assert n_nodes == N_CORES * PER

    dst_core = dst // PER
    dst_slot = dst % PER
    src_chunk = src // PER
    src_slot = src % PER

    # per (core, chunk, slot) in-degree
    deg = np.zeros((N_CORES, N_CORES, SLOTS), dtype=np.int64)
    np.add.at(deg, (dst_core, src_chunk, dst_slot), 1)

    pi = np.argsort(-deg, axis=2, kind="stable")  # [core, chunk, SLOTS]
    deg_sorted = np.take_along_axis(deg, pi, axis=2)
    # shared P per (chunk-pair, rank): max over cores and the pair
    P_rank = deg_sorted.reshape(N_CORES, NP, 2, SLOTS).max(axis=(0, 2))  # [NP, SLOTS]

    calls = []  # per pair: list of (rank_start, n, P)
    for q in range(NP):
        ca = []
        r = 0
        while r < SLOTS and P_rank[q, r] > 0:
            P = int(P_rank[q, r])
            n = min(C // P, SLOTS - r)
            ca.append((r, n, P))
            r += n
        calls.append(ca)
    NCALLS = max(len(ca) for ca in calls)
    infl = sum(n * P for ca in calls for (_, n, P) in ca) * N_CORES * 2 / max(len(src), 1)
    print(f"plan: NCALLS={NCALLS} pad inflation={infl:.3f}")

    # sort edges by (dst_core, src_chunk, dst_slot) for segment extraction
    order = np.lexsort((dst_slot, src_chunk, dst_core))
    s_core = dst_core[order]
    s_chunk = src_chunk[order]
    s_slot = dst_slot[order]
    s_src = src_slot[order].astype(np.int16)
    key = (s_core * N_CORES + s_chunk) * SLOTS + s_slot
    seg_start = np.searchsorted(key, np.arange(N_CORES * N_CORES * SLOTS + 1))

    idx16 = np.full((N_CORES, 128, NCALLS * (C // 16)), PADIDX, dtype=np.int16)
    for cidx in range(N_CORES):
        for k in range(N_CORES):
            q = k // 2
            stream = np.full(NCALLS * C, PADIDX, dtype=np.int16)
            for ci, (r0, n, P) in enumerate(calls[q]):
                base = ci * C
                ranks = pi[cidx, k, r0 : r0 + n]
                degs = deg[cidx, k, ranks]
                kk = (cidx * N_CORES + k) * SLOTS + ranks
                st = seg_start[kk]
                for j in range(n):
                    d = degs[j]
                    if d:
                        stream[base + j * P : base + j * P + d] = s_src[
                            st[j] : st[j] + d
                        ]
            idx16[cidx, 16 * k : 16 * k + 16, :] = stream.reshape(-1, 16).T

    up16 = np.zeros((N_CORES, 128, SLOTS // 16), dtype=np.int16)
    for cidx in range(N_CORES):
        for k in range(N_CORES):
            inv = np.empty(SLOTS, dtype=np.int64)
            inv[pi[cidx, k]] = np.arange(SLOTS)
            up16[cidx, 16 * k : 16 * k + 16, :] = (
                inv.astype(np.int16).reshape(-1, 16).T
            )

    deg_tot = deg.sum(axis=1) + 1  # self loop
    deg_tot[:, PER:] = 1

    pl = _Plan()
    pl.calls = calls
    pl.NCALLS = NCALLS
    pl.idx16 = idx16
    pl.up16 = up16
    pl.deg_tot = deg_tot
    return pl


def _make_in_maps(pl, x, W1, b1, W2, b2, W3, b3, Wf, bf):
    x = np.asarray(x, dtype=np.float32)
    Ws = [np.asarray(W, np.float32) for W in (W1, W2, W3, Wf)]
    bs = [np.asarray(b, np.float32) for b in (b1, b2, b3, bf)]
    # W quadrants: Wq[l, fi_half, fo_half] = W[16*fi:+16, 16*fo:+16]
    Wq = np.zeros((4, 2, 2, 16, 16), dtype=np.float32)
    bq = np.zeros((4, 2, 16, 1), dtype=np.float32)
    for l, (W, b) in enumerate(zip(Ws, bs)):
        Wp = np.zeros((DP, DP), np.float32)
        Wp[: W.shape[0], : W.shape[1]] = W
        for a in range(2):
            for o in range(2):
                Wq[l, a, o] = Wp[16 * a : 16 * a + 16, 16 * o : 16 * o + 16]
        bp = np.zeros(DP, np.float32)
        bp[: b.shape[0]] = b
        bq[l, 0, :, 0] = bp[:16]
        bq[l, 1, :, 0] = bp[16:]
    W_flat = _bf16(Wq.transpose(3, 0, 1, 2, 4).reshape(16, 4 * 2 * 2 * 16))
    b_flat = bq.transpose(2, 0, 1, 3).reshape(16, 8)

    stack8 = np.zeros((128, 16), dtype=np.float32)
    for r in range(128):
        stack8[r, r % 16] = 1.0
    mask6 = np.zeros((16, 1), dtype=np.float32)
    mask6[:6, 0] = 1.0
    ones16 = np.ones((1, 16), dtype=np.float32)

    in_maps = []
    for c in range(N_CORES):
        xT = np.zeros((DP, SLOTS), dtype=np.float32)
        xT[: x.shape[1], :PER] = x[c * PER : (c + 1) * PER].T
        mask = np.zeros((128, 1), dtype=np.float32)
        mask[16 * c : 16 * c + 16, 0] = 1.0
        in_maps.append(
            {
                "xlo_in": _bf16(xT[:16]),
                "xhi_in": _bf16(xT[16:]),
                "idx_in": np.ascontiguousarray(pl.idx16[c]),
                "up_in": np.ascontiguousarray(pl.up16[c]),
                "deg_in": _f16(np.broadcast_to(pl.deg_tot[c][None, :], (16, SLOTS))),
                "W_in": W_flat,
                "b_in": b_flat,
                "s8_in": _bf16(stack8),
                "m6_in": mask6,
                "o16_in": ones16,
                "mask_in": _bf16(mask),
            }
        )
    return in_maps


def _assemble_output(outs_per_core):
    full = np.empty((N_CORES * PER, 6), dtype=np.float32)
    for c in range(N_CORES):
        full[c * PER : (c + 1) * PER] = outs_per_core[c][:, :PER].T
    return full


# ---------------------------------------------------------------------------
# Device kernel
# ---------------------------------------------------------------------------
def _build_kernel(pl):
    NCALLS = pl.NCALLS
    T16 = NCALLS * (C // 16)
    U16 = SLOTS // 16

    nc = bacc.Bacc("TRN2", target_bir_lowering=False, debug=False, num_devices=N_CORES)

    xlo_in = nc.dram_tensor("xlo_in", [16, SLOTS], BF16, kind="ExternalInput")
    xhi_in = nc.dram_tensor("xhi_in", [16, SLOTS], BF16, kind="ExternalInput")
    idx_in = nc.dram_tensor("idx_in", [128, T16], I16, kind="ExternalInput")
    up_in = nc.dram_tensor("up_in", [128, U16], I16, kind="ExternalInput")
    deg_in = nc.dram_tensor("deg_in", [16, SLOTS], F16, kind="ExternalInput")
    W_in = nc.dram_tensor("W_in", [16, 256], BF16, kind="ExternalInput")
    b_in = nc.dram_tensor("b_in", [16, 8], F32, kind="ExternalInput")
    s8_in = nc.dram_tensor("s8_in", [128, 16], BF16, kind="ExternalInput")
    m6_in = nc.dram_tensor("m6_in", [16, 1], F32, kind="ExternalInput")
    o16_in = nc.dram_tensor("o16_in", [1, 16], F32, kind="ExternalInput")
    mask_in = nc.dram_tensor("mask_in", [128, 1], BF16, kind="ExternalInput")
    out_dram = nc.dram_tensor("out", [6, SLOTS], F32, kind="ExternalOutput")

    shard = {}
    tabd = {}
    for k in (1, 2, 3):
        shard[k] = nc.dram_tensor(f"shard{k}", [16, 2 * SLOTS], BF16)
        tabd[k] = nc.dram_tensor(
            f"table{k}", [128, 2 * SLOTS], BF16, addr_space="Shared"
        )
    rgroups = [list(range(N_CORES))]

    echunks = []
    o = 0
    while o < SLOTS:
        w = min(ECH, SLOTS - o)
        echunks.append((o, w))
        o += w
    uchunks = []
    o = 0
    while o < SLOTS:
        w = min(UCH, SLOTS - o)
        uchunks.append((o, w))
        o += w

    with tile.TileContext(nc, num_cores=N_CORES) as tc:
        with (
            tc.tile_pool(name="pers", bufs=1) as pers,
            tc.tile_pool(name="msg", bufs=2) as mpool,
            tc.tile_pool(name="und", bufs=2) as upool,
            tc.tile_pool(name="idxp", bufs=3) as ipool,
            tc.tile_pool(name="chunk", bufs=4) as spool,
            tc.tile_pool(name="row", bufs=1) as rpool,
            tc.tile_pool(name="ps", bufs=6, space="PSUM") as ppool,
            tc.tile_pool(name="ps1", bufs=2, space="PSUM") as ppool1,
        ):
            table = pers.tile([128, 2 * SLOTS], BF16)
            partial = pers.tile([128, 2 * SLOTS], BF16)
            xlo = pers.tile([16, SLOTS], BF16)
            xhi = pers.tile([16, SLOTS], BF16)
            dinvb = pers.tile([16, SLOTS], F16)
            W_sb = pers.tile([16, 256], BF16)
            b_sb = pers.tile([16, 8], F32)
            s8 = pers.tile([128, 16], BF16)
            m6 = pers.tile([16, 1], F32)
            o16 = pers.tile([1, 16], F32)
            maskt = pers.tile([128, 1], BF16)

            nc.sync.dma_start(W_sb[:], W_in[:, :])
            nc.sync.dma_start(b_sb[:], b_in[:, :])
            nc.sync.dma_start(s8[:], s8_in[:, :])
            nc.sync.dma_start(m6[:], m6_in[:, :])
            nc.sync.dma_start(o16[:], o16_in[:, :])
            nc.sync.dma_start(maskt[:], mask_in[:, :])
            nc.sync.dma_start(dinvb[:], deg_in[:, :])
            with nc.allow_low_precision(reason="f16 dinv"):
                nc.vector.reciprocal(dinvb[:], dinvb[:])
            nc.scalar.activation(
                out=dinvb[:], in_=dinvb[:], func=mybir.ActivationFunctionType.Sqrt
            )
            nc.sync.dma_start(xlo[:], xlo_in[:, :])
            nc.sync.dma_start(xhi[:], xhi_in[:, :])
            # prescale the first-layer activations by dinv
            nc.vector.tensor_tensor(
                out=xlo[:], in0=xlo[:], in1=dinvb[:], op=mybir.AluOpType.mult
            )
            nc.vector.tensor_tensor(
                out=xhi[:], in0=xhi[:], in1=dinvb[:], op=mybir.AluOpType.mult
            )

            def Wquad(l, a, o):
                col = ((l * 2 + a) * 2 + o) * 16
                return W_sb[:, col : col + 16]

            def bvec(l, h):
                return b_sb[:, l * 2 + h : l * 2 + h + 1]

            for k in (1, 2, 3):
                l = k - 1
                # ---- GEMM: u = (dinv*x) @ W, quadrant accumulate ----
                for o, w in echunks:
                    uch = spool.tile([16, 2 * ECH], BF16, name=f"u{k}_{o}", tag="chunk")
                    for h in (0, 1):
                        psg = ppool.tile(
                            [16, ECH], F32, space="PSUM", name=f"g{k}_{o}_{h}", tag="ps"
                        )
                        nc.tensor.matmul(
                            psg[:, :w], lhsT=Wquad(l, 0, h), rhs=xlo[:, o : o + w],
                            start=True, stop=False,
                        )
                        nc.tensor.matmul(
                            psg[:, :w], lhsT=Wquad(l, 1, h), rhs=xhi[:, o : o + w],
                            start=False, stop=True,
                        )
                        # cast + interleave write: u_chunk[p, 2j+h] = psg[p, j]
                        nc.vector.tensor_copy(
                            uch[:, h : 2 * w : 2].rearrange("p (w one) -> p w one", one=1),
                            psg[:, :w],
                        )
                    nc.sync.dma_start(shard[k][:, 2 * o : 2 * (o + w)], uch[:, : 2 * w])
                nc.gpsimd.collective_compute(
                    "AllGather",
                    mybir.AluOpType.bypass,
                    replica_groups=rgroups,
                    ins=[shard[k].ap().opt()],
                    outs=[tabd[k].ap().opt()],
                )
                nc.sync.dma_start(table[:], tabd[k].ap())
                nc.vector.memset(partial[:], 0.0)

                # ---- gather + segment reduce ----
                for i in range(NCALLS):
                    it = ipool.tile([128, C // 16], I16, name=f"it{k}_{i}", tag="idx")
                    nc.sync.dma_start(
                        it[:], idx_in[:, i * (C // 16) : (i + 1) * (C // 16)]
                    )
                    msg = mpool.tile([128, 2 * C], BF16, name=f"m{k}_{i}", tag="msg")
                    nc.gpsimd.ap_gather(
                        out_ap=msg[:],
                        in_ap=table[:],
                        idxs_ap=it[:],
                        channels=128,
                        num_elems=SLOTS,
                        d=2,
                        num_idxs=C,
                    )
                    with nc.allow_low_precision(reason="bf16 partials"):
                        for q in range(NP):
                            if i >= len(pl.calls[q]):
                                continue
                            r0, n, P = pl.calls[q][i]
                            nc.vector.tensor_reduce(
                                out=partial[32 * q : 32 * q + 32, 2 * r0 : 2 * (r0 + n)],
                                in_=msg[32 * q : 32 * q + 32, : 2 * n * P].rearrange(
                                    "p (n P two) -> p n two P", P=P, two=2
                                ),
                                axis=mybir.AxisListType.X,
                                op=mybir.AluOpType.add,
                            )

                # ---- unpermute + self-term + chunk-sum + epilogue ----
                for uo, uw in uchunks:
                    iu = ipool.tile([128, UCH // 16], I16, name=f"iu{k}_{uo}", tag="idx")
                    nc.sync.dma_start(
                        iu[:, : uw // 16], up_in[:, uo // 16 : (uo + uw) // 16]
                    )
                    unp = mpool.tile([128, 2 * UCH], BF16, name=f"up{k}_{uo}", tag="msg")
                    nc.gpsimd.ap_gather(
                        out_ap=unp[:, : 2 * uw],
                        in_ap=partial[:],
                        idxs_ap=iu[:, : uw // 16],
                        channels=128,
                        num_elems=SLOTS,
                        d=2,
                        num_idxs=uw,
                    )
                    # unp += mask * u_own (self-loop term, natural order)
                    nc.vector.scalar_tensor_tensor(
                        out=unp[:, : 2 * uw],
                        in0=table[:, 2 * uo : 2 * (uo + uw)],
                        scalar=maskt[:],
                        in1=unp[:, : 2 * uw],
                        op0=mybir.AluOpType.mult,
                        op1=mybir.AluOpType.add,
                    )
                    # deinterleave halves
                    ua = upool.tile([128, UCH], BF16, name=f"ua{k}_{uo}", tag="und")
                    ub = upool.tile([128, UCH], BF16, name=f"ub{k}_{uo}", tag="und")
                    nc.vector.tensor_copy(
                        ua[:, :uw],
                        unp[:, : 2 * uw].rearrange("p (w two) -> p two w", two=2)[
                            :, 0:1, :
                        ],
                    )
                    nc.vector.tensor_copy(
                        ub[:, :uw],
                        unp[:, : 2 * uw].rearrange("p (w two) -> p two w", two=2)[
                            :, 1:2, :
                        ],
                    )
                    for so in range(0, uw, ECH):
                        w = min(ECH, uw - so)
                        o = uo + so
                        for h, uh in ((0, ua), (1, ub)):
                            pss = ppool.tile(
                                [16, ECH], F32, space="PSUM",
                                name=f"s{k}_{o}_{h}", tag="ps",
                            )
                            nc.tensor.matmul(
                                pss[:, :w], lhsT=s8[:], rhs=uh[:, so : so + w],
                                start=True, stop=True,
                            )
                            zc = spool.tile(
                                [16, ECH], BF16, name=f"z{k}_{o}_{h}", tag="chunk"
                            )
                            nc.vector.tensor_copy(zc[:, :w], pss[:, :w])
                            nc.vector.tensor_tensor(
                                out=zc[:, :w], in0=zc[:, :w],
                                in1=dinvb[:, o : o + w], op=mybir.AluOpType.mult,
                            )
                            xh = xlo if h == 0 else xhi
                            nc.scalar.activation(
                                out=xh[:, o : o + w], in_=zc[:, :w],
                                func=mybir.ActivationFunctionType.Relu,
                                bias=bvec(l, h), scale=1.0,
                            )
                            if k < 3:
                                nc.vector.tensor_tensor(
                                    out=xh[:, o : o + w], in0=xh[:, o : o + w],
                                    in1=dinvb[:, o : o + w], op=mybir.AluOpType.mult,
                                )
                nc.vector.memset(xlo[:, PER:SLOTS], 0.0)
                nc.vector.memset(xhi[:, PER:SLOTS], 0.0)

            # ---- final linear + log_softmax (6 outputs live in low half) ----
            for o, w in echunks:
                psg = ppool.tile([16, ECH], F32, space="PSUM", name=f"gf_{o}", tag="ps")
                nc.tensor.matmul(
                    psg[:, :w], lhsT=Wquad(3, 0, 0), rhs=xlo[:, o : o + w],
                    start=True, stop=False,
                )
                nc.tensor.matmul(
                    psg[:, :w], lhsT=Wquad(3, 1, 0), rhs=xhi[:, o : o + w],
                    start=False, stop=True,
                )
                logits = spool.tile([16, ECH], F32, name=f"lg_{o}", tag="chunk")
                nc.vector.tensor_scalar_add(logits[:, :w], psg[:, :w], bvec(3, 0))
                ez = spool.tile([16, ECH], F32, name=f"e_{o}", tag="chunk")
                nc.scalar.activation(
                    out=ez[:, :w], in_=psg[:, :w],
                    func=mybir.ActivationFunctionType.Exp,
                    bias=bvec(3, 0), scale=1.0,
                )
                pss = ppool1.tile([1, ECH], F32, space="PSUM", name=f"sf_{o}", tag="ps1")
                nc.tensor.matmul(
                    pss[:, :w], lhsT=m6[:], rhs=ez[:, :w], start=True, stop=True
                )
                lnr = rpool.tile([1, ECH], F32, name=f"ln_{o}", tag="row")
                nc.scalar.activation(
                    out=lnr[:, :w], in_=pss[:, :w],
                    func=mybir.ActivationFunctionType.Ln,
                )
                psb = ppool.tile([16, ECH], F32, space="PSUM", name=f"bc_{o}", tag="ps")
                nc.tensor.matmul(
                    psb[:, :w], lhsT=o16[:], rhs=lnr[:, :w], start=True, stop=True
                )
                oc = spool.tile([16, ECH], F32, name=f"oc_{o}", tag="chunk")
                nc.vector.tensor_sub(oc[:, :w], logits[:, :w], psb[:, :w])
                nc.sync.dma_start(out_dram[:, o : o + w], oc[0:6, :w])

    nc.compile()
    return nc


# ---------------------------------------------------------------------------
# Entry point
# ---------------------------------------------------------------------------
def kernel(x, edge_index, W1, b1, W2, b2, W3, b3, Wf, bf):
    x = np.asarray(x, dtype=np.float32)
    pl = _build_plan(np.asarray(edge_index), x.shape[0])
    nc = _build_kernel(pl)
    in_maps = _make_in_maps(pl, x, W1, b1, W2, b2, W3, b3, Wf, bf)

    res = run_bass_kernel_spmd(nc, in_maps, core_ids=list(range(N_CORES)))

    LAST_RUN_INFO.clear()
    LAST_RUN_INFO["exec_time_ns"] = res.exec_time_ns
    LAST_RUN_INFO["mean_exec_time_ns"] = res.mean_exec_time_ns

    outs = [res.results[c]["out"] for c in range(N_CORES)]
    return _assemble_output(outs)


# revision 3
# speedup vs baseline: 1.0143x; 1.0023x over previous
"""Trainium2 Bass kernel for 3-layer GCN + Linear + log_softmax (v2).

The baseline spent its whole 33.8ms in DMAGatherAnt descriptor generation
on the single Q7 (gpsimd) engine (~70 cycles/edge-descriptor).  v2 uses the
`ap_gather` SBUF->SBUF ucode instead: measured 27.1ns per index per core,
with the 8 Q7 cores running 8 independent index streams in parallel and
each index moving a full 32-feature edge message (bf16 d=2 feature-pair
rows, 16 partitions x 2 features) => ~3.4 ns/edge.

Layout: feature-major, nodes sharded 8 ways.  Core-chunk k (partitions
[16k,16k+16)) holds the full u table for shard k: table[16k+p, 2n+a] =
feature (16a+p) of shard-k node n, bf16.  Per layer:

  GEMM (quadrant-split W, K=16 accumulate) on dinv-prescaled activations
  -> u chunks -> DRAM shard -> AllGather -> table (straight DMA, the
  AllGather concat IS the table layout) -> NCALLS ap_gather calls of C
  idxs/core (per-core edge streams bucketed by src shard, dst segments
  rank-sorted by per-chunk in-degree; uniform segment length P per
  (chunk-pair, call)) -> strided tensor_reduce into bf16 partial[rank]
  -> unpermute (ap_gather d=2 over partial) -> masked add of the
  self-loop u_own term -> stacked-identity PE matmuls sum the 8 chunks
  -> z = dinv*S, x' = relu(z+b), prescale by dinv for the next GEMM.

Final layer: logits feature-major; log_softmax = logits - ones^T @
ln(mask6^T @ exp(logits)) via two tiny matmuls.  Host de-shards.
"""

import math

import numpy as np

from concourse import bass, mybir, bacc, tile
from concourse.bass_utils import run_bass_kernel_spmd

F32 = mybir.dt.float32
F16 = mybir.dt.float16
BF16 = mybir.dt.bfloat16
I16 = mybir.dt.int16

N_CORES = 8
PER = 12500
SLOTS = 12544
NP = 4  # chunk pairs
DP = 32
DIMS = (16, 32, 24, 12, 6)
C = 2048  # gather-call indices per core
UCH = 2048  # unpermute chunk columns
ECH = 512  # epilogue / GEMM chunk
PADIDX = SLOTS - 1  # dummy node slot (always-zero table column)

LAST_RUN_INFO = {}


def _bf16(a):
    import ml_dtypes

    return np.asarray(a).astype(ml_dtypes.bfloat16)


def _f16(a):
    return np.asarray(a).astype(np.float16)


class _Plan:
    pass


# ---------------------------------------------------------------------------
# Host-side plan (pure integer index manipulation)
# ---------------------------------------------------------------------------
def _build_plan(edge_index, n_nodes):
    src = np.asarray(edge_index[0], dtype=np.int64)
    dst = np.asarray(edge_index[1], dtype=np.int64)
    assert n_nodes == N_CORES * PER

    dst_core = dst // PER
    dst_slot = dst % PER
    src_chunk = src // PER
    src_slot = src % PER

    # per (core, chunk, slot) in-degree
    deg = np.zeros((N_CORES, N_CORES, SLOTS), dtype=np.int64)
    np.add.at(deg, (dst_core, src_chunk, dst_slot), 1)

    pi = np.argsort(-deg, axis=2, kind="stable")  # [core, chunk, SLOTS]
    deg_sorted = np.take_along_axis(deg, pi, axis=2)
    # shared P per (chunk-pair, rank): max over cores and the pair
    P_rank = deg_sorted.reshape(N_CORES, NP, 2, SLOTS).max(axis=(0, 2))  # [NP, SLOTS]

    calls = []  # per pair: list of (rank_start, n, P)
    for q in range(NP):
        ca = []
        r = 0
        while r < SLOTS and P_rank[q, r] > 0:
            P = int(P_rank[q, r])
            n = min(C // P, SLOTS - r)
            ca.append((r, n, P))
            r += n
        calls.append(ca)
    NCALLS = max(len(ca) for ca in calls)
    infl = sum(n * P for ca in calls for (_, n, P) in ca) * N_CORES * 2 / max(len(src), 1)
    print(f"plan: NCALLS={NCALLS} pad inflation={infl:.3f}")

    # sort edges by (dst_core, src_chunk, dst_slot) for segment extraction
    order = np.lexsort((dst_slot, src_chunk, dst_core))
    s_core = dst_core[order]
    s_chunk = src_chunk[order]
    s_slot = dst_slot[order]
    s_src = src_slot[order].astype(np.int16)
    key = (s_core * N_CORES + s_chunk) * SLOTS + s_slot
    seg_start = np.searchsorted(key, np.arange(N_CORES * N_CORES * SLOTS + 1))

    idx16 = np.full((N_CORES, 128, NCALLS * (C // 16)), PADIDX, dtype=np.int16)
    for cidx in range(N_CORES):
        for k in range(N_CORES):
            q = k // 2
            stream = np.full(NCALLS * C, PADIDX, dtype=np.int16)
            for ci, (r0, n, P) in enumerate(calls[q]):
                base = ci * C
                ranks = pi[cidx, k, r0 : r0 + n]
                degs = deg[cidx, k, ranks]
                kk = (cidx * N_CORES + k) * SLOTS + ranks
                st = seg_start[kk]
                for j in range(n):
                    d = degs[j]
                    if d:
                        stream[base + j * P : base + j * P + d] = s_src[
                            st[j] : st[j] + d
                        ]
            idx16[cidx, 16 * k : 16 * k + 16, :] = stream.reshape(-1, 16).T

    up16 = np.zeros((N_CORES, 128, SLOTS // 16), dtype=np.int16)
    for cidx in range(N_CORES):
        for k in range(N_CORES):
            inv = np.empty(SLOTS, dtype=np.int64)
            inv[pi[cidx, k]] = np.arange(SLOTS)
            up16[cidx, 16 * k : 16 * k + 16, :] = (
                inv.astype(np.int16).reshape(-1, 16).T
            )

    deg_tot = deg.sum(axis=1) + 1  # self loop
    deg_tot[:, PER:] = 1

    pl = _Plan()
    pl.calls = calls
    pl.NCALLS = NCALLS
    pl.idx16 = idx16
    pl.up16 = up16
    pl.deg_tot = deg_tot
    return pl


def _make_in_maps(pl, x, W1, b1, W2, b2, W3, b3, Wf, bf):
    x = np.asarray(x, dtype=np.float32)
    Ws = [np.asarray(W, np.float32) for W in (W1, W2, W3, Wf)]
    bs = [np.asarray(b, np.float32) for b in (b1, b2, b3, bf)]
    # W quadrants: Wq[l, fi_half, fo_half] = W[16*fi:+16, 16*fo:+16]
    Wq = np.zeros((4, 2, 2, 16, 16), dtype=np.float32)
    bq = np.zeros((4, 2, 16, 1), dtype=np.float32)
    for l, (W, b) in enumerate(zip(Ws, bs)):
        Wp = np.zeros((DP, DP), np.float32)
        Wp[: W.shape[0], : W.shape[1]] = W
        for a in range(2):
            for o in range(2):
                Wq[l, a, o] = Wp[16 * a : 16 * a + 16, 16 * o : 16 * o + 16]
        bp = np.zeros(DP, np.float32)
        bp[: b.shape[0]] = b
        bq[l, 0, :, 0] = bp[:16]
        bq[l, 1, :, 0] = bp[16:]
    W_flat = _bf16(Wq.transpose(3, 0, 1, 2, 4).reshape(16, 4 * 2 * 2 * 16))
    b_flat = bq.transpose(2, 0, 1, 3).reshape(16, 8)

    stack8 = np.zeros((128, 16), dtype=np.float32)
    for r in range(128):
        stack8[r, r % 16] = 1.0
    mask6 = np.zeros((16, 1), dtype=np.float32)
    mask6[:6, 0] = 1.0
    ones16 = np.ones((1, 16), dtype=np.float32)

    in_maps = []
    for c in range(N_CORES):
        xT = np.zeros((DP, SLOTS), dtype=np.float32)
        xT[: x.shape[1], :PER] = x[c * PER : (c + 1) * PER].T
        mask = np.zeros((128, 1), dtype=np.float32)
        mask[16 * c : 16 * c + 16, 0] = 1.0
        in_maps.append(
            {
                "xlo_in": _bf16(xT[:16]),
                "xhi_in": _bf16(xT[16:]),
                "idx_in": np.ascontiguousarray(pl.idx16[c]),
                "up_in": np.ascontiguousarray(pl.up16[c]),
                "deg_in": _f16(np.broadcast_to(pl.deg_tot[c][None, :], (16, SLOTS))),
                "W_in": W_flat,
                "b_in": b_flat,
                "s8_in": _bf16(stack8),
                "m6_in": mask6,
                "o16_in": ones16,
                "mask_in": _bf16(mask),
            }
        )
    return in_maps


def _assemble_output(outs_per_core):
    full = np.empty((N_CORES * PER, 6), dtype=np.float32)
    for c in range(N_CORES):
        full[c * PER : (c + 1) * PER] = outs_per_core[c][:, :PER].T
    return full


# ---------------------------------------------------------------------------
# Device kernel
# ---------------------------------------------------------------------------
def _build_kernel(pl):
    NCALLS = pl.NCALLS
    T16 = NCALLS * (C // 16)
    U16 = SLOTS // 16

    nc = bacc.Bacc("TRN2", target_bir_lowering=False, debug=False, num_devices=N_CORES)

    xlo_in = nc.dram_tensor("xlo_in", [16, SLOTS], BF16, kind="ExternalInput")
    xhi_in = nc.dram_tensor("xhi_in", [16, SLOTS], BF16, kind="ExternalInput")
    idx_in = nc.dram_tensor("idx_in", [128, T16], I16, kind="ExternalInput")
    up_in = nc.dram_tensor("up_in", [128, U16], I16, kind="ExternalInput")
    deg_in = nc.dram_tensor("deg_in", [16, SLOTS], F16, kind="ExternalInput")
    W_in = nc.dram_tensor("W_in", [16, 256], BF16, kind="ExternalInput")
    b_in = nc.dram_tensor("b_in", [16, 8], F32, kind="ExternalInput")
    s8_in = nc.dram_tensor("s8_in", [128, 16], BF16, kind="ExternalInput")
    m6_in = nc.dram_tensor("m6_in", [16, 1], F32, kind="ExternalInput")
    o16_in = nc.dram_tensor("o16_in", [1, 16], F32, kind="ExternalInput")
    mask_in = nc.dram_tensor("mask_in", [128, 1], BF16, kind="ExternalInput")
    out_dram = nc.dram_tensor("out", [6, SLOTS], F32, kind="ExternalOutput")

    shard = {}
    tabd = {}
    for k in (1, 2, 3):
        shard[k] = nc.dram_tensor(f"shard{k}", [16, 2 * SLOTS], BF16)
        tabd[k] = nc.dram_tensor(
            f"table{k}", [128, 2 * SLOTS], BF16, addr_space="Shared"
        )
    rgroups = [list(range(N_CORES))]

    echunks = []
    o = 0
    while o < SLOTS:
        w = min(ECH, SLOTS - o)
        echunks.append((o, w))
        o += w
    uchunks = []
    o = 0
    while o < SLOTS:
        w = min(UCH, SLOTS - o)
        uchunks.append((o, w))
        o += w

    with tile.TileContext(nc, num_cores=N_CORES) as tc:
        with (
            tc.tile_pool(name="pers", bufs=1) as pers,
            tc.tile_pool(name="msg", bufs=2) as mpool,
            tc.tile_pool(name="und", bufs=2) as upool,
            tc.tile_pool(name="idxp", bufs=3) as ipool,
            tc.tile_pool(name="chunk", bufs=4) as spool,
            tc.tile_pool(name="row", bufs=1) as rpool,
            tc.tile_pool(name="ps", bufs=6, space="PSUM") as ppool,
            tc.tile_pool(name="ps1", bufs=2, space="PSUM") as ppool1,
        ):
            table = pers.tile([128, 2 * SLOTS], BF16)
            partial = pers.tile([128, 2 * SLOTS], BF16)
            xlo = pers.tile([16, SLOTS], BF16)
            xhi = pers.tile([16, SLOTS], BF16)
            dinvb = pers.tile([16, SLOTS], F16)
            W_sb = pers.tile([16, 256], BF16)
            b_sb = pers.tile([16, 8], F32)
            s8 = pers.tile([128, 16], BF16)
            m6 = pers.tile([16, 1], F32)
            o16 = pers.tile([1, 16], F32)
            maskt = pers.tile([128, 1], BF16)

            nc.sync.dma_start(W_sb[:], W_in[:, :])
            nc.sync.dma_start(b_sb[:], b_in[:, :])
            nc.sync.dma_start(s8[:], s8_in[:, :])
            nc.sync.dma_start(m6[:], m6_in[:, :])
            nc.sync.dma_start(o16[:], o16_in[:, :])
            nc.sync.dma_start(maskt[:], mask_in[:, :])
            nc.sync.dma_start(dinvb[:], deg_in[:, :])
            with nc.allow_low_precision(reason="f16 dinv"):
                nc.vector.reciprocal(dinvb[:], dinvb[:])
            nc.scalar.activation(
                out=dinvb[:], in_=dinvb[:], func=mybir.ActivationFunctionType.Sqrt
            )
            nc.sync.dma_start(xlo[:], xlo_in[:, :])
            nc.sync.dma_start(xhi[:], xhi_in[:, :])
            # prescale the first-layer activations by dinv
            nc.vector.tensor_tensor(
                out=xlo[:], in0=xlo[:], in1=dinvb[:], op=mybir.AluOpType.mult
            )
            nc.vector.tensor_tensor(
                out=xhi[:], in0=xhi[:], in1=dinvb[:], op=mybir.AluOpType.mult
            )

            def Wquad(l, a, o):
                col = ((l * 2 + a) * 2 + o) * 16
                return W_sb[:, col : col + 16]

            def bvec(l, h):
                return b_sb[:, l * 2 + h : l * 2 + h + 1]

            def emit_gemm(kk, o, w):
                # u_kk = (dinv*x_kk) @ W_kk for columns [o, o+w) -> shard[kk]
                l = kk - 1
                uch = spool.tile([16, 2 * ECH], BF16, name=f"u{kk}_{o}", tag="chunk")
                for h in (0, 1):
                    psg = ppool.tile(
                        [16, ECH], F32, space="PSUM", name=f"g{kk}_{o}_{h}", tag="ps"
                    )
                    nc.tensor.matmul(
                        psg[:, :w], lhsT=Wquad(l, 0, h), rhs=xlo[:, o : o + w],
                        start=True, stop=False,
                    )
                    nc.tensor.matmul(
                        psg[:, :w], lhsT=Wquad(l, 1, h), rhs=xhi[:, o : o + w],
                        start=False, stop=True,
                    )
                    # cast + interleave write: u_chunk[p, 2j+h] = psg[p, j]
                    nc.vector.tensor_copy(
                        uch[:, h : 2 * w : 2].rearrange("p (w one) -> p w one", one=1),
                        psg[:, :w],
                    )
                nc.sync.dma_start(shard[kk][:, 2 * o : 2 * (o + w)], uch[:, : 2 * w])

            def emit_final(o, w):
                # logits + log_softmax for columns [o, o+w) -> out_dram
                psg = ppool.tile([16, ECH], F32, space="PSUM", name=f"gf_{o}", tag="ps")
                nc.tensor.matmul(
                    psg[:, :w], lhsT=Wquad(3, 0, 0), rhs=xlo[:, o : o + w],
                    start=True, stop=False,
                )
                nc.tensor.matmul(
                    psg[:, :w], lhsT=Wquad(3, 1, 0), rhs=xhi[:, o : o + w],
                    start=False, stop=True,
                )
                logits = spool.tile([16, ECH], F32, name=f"lg_{o}", tag="chunk")
                nc.vector.tensor_scalar_add(logits[:, :w], psg[:, :w], bvec(3, 0))
                ez = spool.tile([16, ECH], F32, name=f"e_{o}", tag="chunk")
                nc.scalar.activation(
                    out=ez[:, :w], in_=psg[:, :w],
                    func=mybir.ActivationFunctionType.Exp,
                    bias=bvec(3, 0), scale=1.0,
                )
                pss = ppool1.tile([1, ECH], F32, space="PSUM", name=f"sf_{o}", tag="ps1")
                nc.tensor.matmul(
                    pss[:, :w], lhsT=m6[:], rhs=ez[:, :w], start=True, stop=True
                )
                lnr = rpool.tile([1, ECH], F32, name=f"ln_{o}", tag="row")
                nc.scalar.activation(
                    out=lnr[:, :w], in_=pss[:, :w],
                    func=mybir.ActivationFunctionType.Ln,
                )
                psb = ppool.tile([16, ECH], F32, space="PSUM", name=f"bc_{o}", tag="ps")
                nc.tensor.matmul(
                    psb[:, :w], lhsT=o16[:], rhs=lnr[:, :w], start=True, stop=True
                )
                oc = spool.tile([16, ECH], F32, name=f"oc_{o}", tag="chunk")
                nc.vector.tensor_sub(oc[:, :w], logits[:, :w], psb[:, :w])
                nc.sync.dma_start(out_dram[:, o : o + w], oc[0:6, :w])

            # layer-1 GEMM from the (prescaled) input activations
            for o, w in echunks:
                emit_gemm(1, o, w)

            for k in (1, 2, 3):
                l = k - 1
                nc.gpsimd.collective_compute(
                    "AllGather",
                    mybir.AluOpType.bypass,
                    replica_groups=rgroups,
                    ins=[shard[k].ap().opt()],
                    outs=[tabd[k].ap().opt()],
                )
                nc.sync.dma_start(table[:], tabd[k].ap())
                nc.vector.memset(partial[:], 0.0)

                # ---- gather + segment reduce ----
                for i in range(NCALLS):
                    it = ipool.tile([128, C // 16], I16, name=f"it{k}_{i}", tag="idx")
                    nc.sync.dma_start(
                        it[:], idx_in[:, i * (C // 16) : (i + 1) * (C // 16)]
                    )
                    msg = mpool.tile([128, 2 * C], BF16, name=f"m{k}_{i}", tag="msg")
                    nc.gpsimd.ap_gather(
                        out_ap=msg[:],
                        in_ap=table[:],
                        idxs_ap=it[:],
                        channels=128,
                        num_elems=SLOTS,
                        d=2,
                        num_idxs=C,
                    )
                    with nc.allow_low_precision(reason="bf16 partials"):
                        for q in range(NP):
                            if i >= len(pl.calls[q]):
                                continue
                            r0, n, P = pl.calls[q][i]
                            nc.vector.tensor_reduce(
                                out=partial[32 * q : 32 * q + 32, 2 * r0 : 2 * (r0 + n)],
                                in_=msg[32 * q : 32 * q + 32, : 2 * n * P].rearrange(
                                    "p (n P two) -> p n two P", P=P, two=2
                                ),
                                axis=mybir.AxisListType.X,
                                op=mybir.AluOpType.add,
                            )

                # ---- unpermute + self-term + chunk-sum + epilogue ----
                for uo, uw in uchunks:
                    iu = ipool.tile([128, UCH // 16], I16, name=f"iu{k}_{uo}", tag="idx")
                    nc.sync.dma_start(
                        iu[:, : uw // 16], up_in[:, uo // 16 : (uo + uw) // 16]
                    )
                    unp = mpool.tile([128, 2 * UCH], BF16, name=f"up{k}_{uo}", tag="msg")
                    nc.gpsimd.ap_gather(
                        out_ap=unp[:, : 2 * uw],
                        in_ap=partial[:],
                        idxs_ap=iu[:, : uw // 16],
                        channels=128,
                        num_elems=SLOTS,
                        d=2,
                        num_idxs=uw,
                    )
                    # unp += mask * u_own (self-loop term, natural order)
                    nc.vector.scalar_tensor_tensor(
                        out=unp[:, : 2 * uw],
                        in0=table[:, 2 * uo : 2 * (uo + uw)],
                        scalar=maskt[:],
                        in1=unp[:, : 2 * uw],
                        op0=mybir.AluOpType.mult,
                        op1=mybir.AluOpType.add,
                    )
                    # deinterleave halves
                    ua = upool.tile([128, UCH], BF16, name=f"ua{k}_{uo}", tag="und")
                    ub = upool.tile([128, UCH], BF16, name=f"ub{k}_{uo}", tag="und")
                    nc.vector.tensor_copy(
                        ua[:, :uw],
                        unp[:, : 2 * uw].rearrange("p (w two) -> p two w", two=2)[
                            :, 0:1, :
                        ],
                    )
                    nc.vector.tensor_copy(
                        ub[:, :uw],
                        unp[:, : 2 * uw].rearrange("p (w two) -> p two w", two=2)[
                            :, 1:2, :
                        ],
                    )
                    for so in range(0, uw, ECH):
                        w = min(ECH, uw - so)
                        o = uo + so
                        for h, uh in ((0, ua), (1, ub)):
                            pss = ppool.tile(
                                [16, ECH], F32, space="PSUM",
                                name=f"s{k}_{o}_{h}", tag="ps",
                            )
                            nc.tensor.matmul(
                                pss[:, :w], lhsT=s8[:], rhs=uh[:, so : so + w],
                                start=True, stop=True,
                            )
                            zc = spool.tile(
                                [16, ECH], BF16, name=f"z{k}_{o}_{h}", tag="chunk"
                            )
                            nc.vector.tensor_copy(zc[:, :w], pss[:, :w])
                            nc.vector.tensor_tensor(
                                out=zc[:, :w], in0=zc[:, :w],
                                in1=dinvb[:, o : o + w], op=mybir.AluOpType.mult,
                            )
                            xh = xlo if h == 0 else xhi
                            nc.scalar.activation(
                                out=xh[:, o : o + w], in_=zc[:, :w],
                                func=mybir.ActivationFunctionType.Relu,
                                bias=bvec(l, h), scale=1.0,
                            )
                            if k < 3:
                                nc.vector.tensor_tensor(
                                    out=xh[:, o : o + w], in0=xh[:, o : o + w],
                                    in1=dinvb[:, o : o + w], op=mybir.AluOpType.mult,
                                )
                        # fused next-stage for these freshly-written columns:
                        # hides under the remaining unperm gather calls
                        if k < 3:
                            if o + w > PER:
                                nc.vector.memset(xlo[:, PER:SLOTS], 0.0)
                                nc.vector.memset(xhi[:, PER:SLOTS], 0.0)
                            emit_gemm(k + 1, o, w)
                        else:
                            emit_final(o, w)

    nc.compile()
    return nc


# ---------------------------------------------------------------------------
# Entry point
# ---------------------------------------------------------------------------
def kernel(x, edge_index, W1, b1, W2, b2, W3, b3, Wf, bf):
    x = np.asarray(x, dtype=np.float32)
    pl = _build_plan(np.asarray(edge_index), x.shape[0])
    nc = _build_kernel(pl)
    in_maps = _make_in_maps(pl, x, W1, b1, W2, b2, W3, b3, Wf, bf)

    res = run_bass_kernel_spmd(nc, in_maps, core_ids=list(range(N_CORES)))

    LAST_RUN_INFO.clear()
    LAST_RUN_INFO["exec_time_ns"] = res.exec_time_ns
    LAST_RUN_INFO["mean_exec_time_ns"] = res.mean_exec_time_ns

    outs = [res.results[c]["out"] for c in range(N_CORES)]
    return _assemble_output(outs)


# revision 4
# speedup vs baseline: 1.0159x; 1.0015x over previous
"""Trainium2 Bass kernel for 3-layer GCN + Linear + log_softmax (v2).

The baseline spent its whole 33.8ms in DMAGatherAnt descriptor generation
on the single Q7 (gpsimd) engine (~70 cycles/edge-descriptor).  v2 uses the
`ap_gather` SBUF->SBUF ucode instead: measured 27.1ns per index per core,
with the 8 Q7 cores running 8 independent index streams in parallel and
each index moving a full 32-feature edge message (bf16 d=2 feature-pair
rows, 16 partitions x 2 features) => ~3.4 ns/edge.

Layout: feature-major, nodes sharded 8 ways.  Core-chunk k (partitions
[16k,16k+16)) holds the full u table for shard k: table[16k+p, 2n+a] =
feature (16a+p) of shard-k node n, bf16.  Per layer:

  GEMM (quadrant-split W, K=16 accumulate) on dinv-prescaled activations
  -> u chunks -> DRAM shard -> AllGather -> table (straight DMA, the
  AllGather concat IS the table layout) -> NCALLS ap_gather calls of C
  idxs/core (per-core edge streams bucketed by src shard, dst segments
  rank-sorted by per-chunk in-degree; uniform segment length P per
  (chunk-pair, call)) -> strided tensor_reduce into bf16 partial[rank]
  -> unpermute (ap_gather d=2 over partial) -> masked add of the
  self-loop u_own term -> stacked-identity PE matmuls sum the 8 chunks
  -> z = dinv*S, x' = relu(z+b), prescale by dinv for the next GEMM.

Final layer: logits feature-major; log_softmax = logits - ones^T @
ln(mask6^T @ exp(logits)) via two tiny matmuls.  Host de-shards.
"""

import math

import numpy as np

from concourse import bass, mybir, bacc, tile
from concourse.bass_utils import run_bass_kernel_spmd

F32 = mybir.dt.float32
F16 = mybir.dt.float16
BF16 = mybir.dt.bfloat16
I16 = mybir.dt.int16

N_CORES = 8
PER = 12500
SLOTS = 12544
NP = 4  # chunk pairs
DP = 32
DIMS = (16, 32, 24, 12, 6)
C = 2048  # gather-call indices per core
UCH = 2048  # unpermute chunk columns
ECH = 512  # epilogue / GEMM chunk
PADIDX = SLOTS - 1  # dummy node slot (always-zero table column)

LAST_RUN_INFO = {}


def _bf16(a):
    import ml_dtypes

    return np.asarray(a).astype(ml_dtypes.bfloat16)


def _f16(a):
    return np.asarray(a).astype(np.float16)


class _Plan:
    pass


# ---------------------------------------------------------------------------
# Host-side plan (pure integer index manipulation)
# ---------------------------------------------------------------------------
def _build_plan(edge_index, n_nodes):
    src = np.asarray(edge_index[0], dtype=np.int64)
    dst = np.asarray(edge_index[1], dtype=np.int64)
    assert n_nodes == N_CORES * PER

    dst_core = dst // PER
    dst_slot = dst % PER
    src_chunk = src // PER
    src_slot = src % PER

    # per (core, chunk, slot) in-degree
    deg = np.zeros((N_CORES, N_CORES, SLOTS), dtype=np.int64)
    np.add.at(deg, (dst_core, src_chunk, dst_slot), 1)

    pi = np.argsort(-deg, axis=2, kind="stable")  # [core, chunk, SLOTS]
    deg_sorted = np.take_along_axis(deg, pi, axis=2)
    # shared P per (chunk-pair, rank): max over cores and the pair
    P_rank = deg_sorted.reshape(N_CORES, NP, 2, SLOTS).max(axis=(0, 2))  # [NP, SLOTS]

    calls = []  # per pair: list of (rank_start, n, P)
    for q in range(NP):
        ca = []
        r = 0
        while r < SLOTS and P_rank[q, r] > 0:
            P = int(P_rank[q, r])
            n = min(C // P, SLOTS - r)
            ca.append((r, n, P))
            r += n
        calls.append(ca)
    NCALLS = max(len(ca) for ca in calls)
    infl = sum(n * P for ca in calls for (_, n, P) in ca) * N_CORES * 2 / max(len(src), 1)
    print(f"plan: NCALLS={NCALLS} pad inflation={infl:.3f}")

    # sort edges by (dst_core, src_chunk, dst_slot) for segment extraction
    order = np.lexsort((dst_slot, src_chunk, dst_core))
    s_core = dst_core[order]
    s_chunk = src_chunk[order]
    s_slot = dst_slot[order]
    s_src = src_slot[order].astype(np.int16)
    key = (s_core * N_CORES + s_chunk) * SLOTS + s_slot
    seg_start = np.searchsorted(key, np.arange(N_CORES * N_CORES * SLOTS + 1))

    idx16 = np.full((N_CORES, 128, NCALLS * (C // 16)), PADIDX, dtype=np.int16)
    for cidx in range(N_CORES):
        for k in range(N_CORES):
            q = k // 2
            stream = np.full(NCALLS * C, PADIDX, dtype=np.int16)
            for ci, (r0, n, P) in enumerate(calls[q]):
                base = ci * C
                ranks = pi[cidx, k, r0 : r0 + n]
                degs = deg[cidx, k, ranks]
                kk = (cidx * N_CORES + k) * SLOTS + ranks
                st = seg_start[kk]
                for j in range(n):
                    d = degs[j]
                    if d:
                        stream[base + j * P : base + j * P + d] = s_src[
                            st[j] : st[j] + d
                        ]
            idx16[cidx, 16 * k : 16 * k + 16, :] = stream.reshape(-1, 16).T

    up16 = np.zeros((N_CORES, 128, SLOTS // 16), dtype=np.int16)
    for cidx in range(N_CORES):
        for k in range(N_CORES):
            inv = np.empty(SLOTS, dtype=np.int64)
            inv[pi[cidx, k]] = np.arange(SLOTS)
            up16[cidx, 16 * k : 16 * k + 16, :] = (
                inv.astype(np.int16).reshape(-1, 16).T
            )

    deg_tot = deg.sum(axis=1) + 1  # self loop
    deg_tot[:, PER:] = 1

    pl = _Plan()
    pl.calls = calls
    pl.NCALLS = NCALLS
    pl.idx16 = idx16
    pl.up16 = up16
    pl.deg_tot = deg_tot
    return pl


def _make_in_maps(pl, x, W1, b1, W2, b2, W3, b3, Wf, bf):
    x = np.asarray(x, dtype=np.float32)
    Ws = [np.asarray(W, np.float32) for W in (W1, W2, W3, Wf)]
    bs = [np.asarray(b, np.float32) for b in (b1, b2, b3, bf)]
    # W quadrants: Wq[l, fi_half, fo_half] = W[16*fi:+16, 16*fo:+16]
    Wq = np.zeros((4, 2, 2, 16, 16), dtype=np.float32)
    bq = np.zeros((4, 2, 16, 1), dtype=np.float32)
    for l, (W, b) in enumerate(zip(Ws, bs)):
        Wp = np.zeros((DP, DP), np.float32)
        Wp[: W.shape[0], : W.shape[1]] = W
        for a in range(2):
            for o in range(2):
                Wq[l, a, o] = Wp[16 * a : 16 * a + 16, 16 * o : 16 * o + 16]
        bp = np.zeros(DP, np.float32)
        bp[: b.shape[0]] = b
        bq[l, 0, :, 0] = bp[:16]
        bq[l, 1, :, 0] = bp[16:]
    W_flat = _bf16(Wq.transpose(3, 0, 1, 2, 4).reshape(16, 4 * 2 * 2 * 16))
    b_flat = bq.transpose(2, 0, 1, 3).reshape(16, 8)

    stack8 = np.zeros((128, 16), dtype=np.float32)
    for r in range(128):
        stack8[r, r % 16] = 1.0
    mask6 = np.zeros((16, 1), dtype=np.float32)
    mask6[:6, 0] = 1.0
    ones16 = np.ones((1, 16), dtype=np.float32)

    in_maps = []
    for c in range(N_CORES):
        xT = np.zeros((DP, SLOTS), dtype=np.float32)
        xT[: x.shape[1], :PER] = x[c * PER : (c + 1) * PER].T
        mask = np.zeros((128, 1), dtype=np.float32)
        mask[16 * c : 16 * c + 16, 0] = 1.0
        in_maps.append(
            {
                "xlo_in": _bf16(xT[:16]),
                "xhi_in": _bf16(xT[16:]),
                "idx_in": np.ascontiguousarray(pl.idx16[c]),
                "up_in": np.ascontiguousarray(pl.up16[c]),
                "deg_in": _f16(np.broadcast_to(pl.deg_tot[c][None, :], (16, SLOTS))),
                "W_in": W_flat,
                "b_in": b_flat,
                "s8_in": _bf16(stack8),
                "m6_in": mask6,
                "o16_in": ones16,
                "mask_in": _bf16(mask),
            }
        )
    return in_maps


def _assemble_output(outs_per_core):
    full = np.empty((N_CORES * PER, 6), dtype=np.float32)
    for c in range(N_CORES):
        full[c * PER : (c + 1) * PER] = outs_per_core[c][:, :PER].T
    return full


# ---------------------------------------------------------------------------
# Device kernel
# ---------------------------------------------------------------------------
def _build_kernel(pl):
    NCALLS = pl.NCALLS
    T16 = NCALLS * (C // 16)
    U16 = SLOTS // 16

    nc = bacc.Bacc("TRN2", target_bir_lowering=False, debug=False, num_devices=N_CORES)

    xlo_in = nc.dram_tensor("xlo_in", [16, SLOTS], BF16, kind="ExternalInput")
    xhi_in = nc.dram_tensor("xhi_in", [16, SLOTS], BF16, kind="ExternalInput")
    idx_in = nc.dram_tensor("idx_in", [128, T16], I16, kind="ExternalInput")
    up_in = nc.dram_tensor("up_in", [128, U16], I16, kind="ExternalInput")
    deg_in = nc.dram_tensor("deg_in", [16, SLOTS], F16, kind="ExternalInput")
    W_in = nc.dram_tensor("W_in", [16, 256], BF16, kind="ExternalInput")
    b_in = nc.dram_tensor("b_in", [16, 8], F32, kind="ExternalInput")
    s8_in = nc.dram_tensor("s8_in", [128, 16], BF16, kind="ExternalInput")
    m6_in = nc.dram_tensor("m6_in", [16, 1], F32, kind="ExternalInput")
    o16_in = nc.dram_tensor("o16_in", [1, 16], F32, kind="ExternalInput")
    mask_in = nc.dram_tensor("mask_in", [128, 1], BF16, kind="ExternalInput")
    out_dram = nc.dram_tensor("out", [6, SLOTS], F32, kind="ExternalOutput")

    SPL = 8192
    shardA = {}
    shardB = {}
    tabdA = {}
    tabdB = {}
    for k in (1, 2, 3):
        shardA[k] = nc.dram_tensor(f"shardA{k}", [16, 2 * SPL], BF16)
        shardB[k] = nc.dram_tensor(f"shardB{k}", [16, 2 * (SLOTS - SPL)], BF16)
        tabdA[k] = nc.dram_tensor(
            f"tableA{k}", [128, 2 * SPL], BF16, addr_space="Shared"
        )
        tabdB[k] = nc.dram_tensor(
            f"tableB{k}", [128, 2 * (SLOTS - SPL)], BF16, addr_space="Shared"
        )
    rgroups = [list(range(N_CORES))]

    echunks = []
    o = 0
    while o < SLOTS:
        w = min(ECH, SLOTS - o)
        echunks.append((o, w))
        o += w
    uchunks = []
    o = 0
    while o < SLOTS:
        w = min(UCH, SLOTS - o)
        uchunks.append((o, w))
        o += w

    with tile.TileContext(nc, num_cores=N_CORES) as tc:
        with (
            tc.tile_pool(name="pers", bufs=1) as pers,
            tc.tile_pool(name="msg", bufs=2) as mpool,
            tc.tile_pool(name="und", bufs=2) as upool,
            tc.tile_pool(name="idxp", bufs=3) as ipool,
            tc.tile_pool(name="chunk", bufs=4) as spool,
            tc.tile_pool(name="row", bufs=1) as rpool,
            tc.tile_pool(name="ps", bufs=6, space="PSUM") as ppool,
            tc.tile_pool(name="ps1", bufs=2, space="PSUM") as ppool1,
        ):
            table = pers.tile([128, 2 * SLOTS], BF16)
            partial = pers.tile([128, 2 * SLOTS], BF16)
            xlo = pers.tile([16, SLOTS], BF16)
            xhi = pers.tile([16, SLOTS], BF16)
            dinvb = pers.tile([16, SLOTS], F16)
            W_sb = pers.tile([16, 256], BF16)
            b_sb = pers.tile([16, 8], F32)
            s8 = pers.tile([128, 16], BF16)
            m6 = pers.tile([16, 1], F32)
            o16 = pers.tile([1, 16], F32)
            maskt = pers.tile([128, 1], BF16)

            nc.sync.dma_start(W_sb[:], W_in[:, :])
            nc.sync.dma_start(b_sb[:], b_in[:, :])
            nc.sync.dma_start(s8[:], s8_in[:, :])
            nc.sync.dma_start(m6[:], m6_in[:, :])
            nc.sync.dma_start(o16[:], o16_in[:, :])
            nc.sync.dma_start(maskt[:], mask_in[:, :])
            nc.sync.dma_start(dinvb[:], deg_in[:, :])
            with nc.allow_low_precision(reason="f16 dinv"):
                nc.vector.reciprocal(dinvb[:], dinvb[:])
            nc.scalar.activation(
                out=dinvb[:], in_=dinvb[:], func=mybir.ActivationFunctionType.Sqrt
            )
            nc.sync.dma_start(xlo[:], xlo_in[:, :])
            nc.sync.dma_start(xhi[:], xhi_in[:, :])
            # prescale the first-layer activations by dinv
            nc.vector.tensor_tensor(
                out=xlo[:], in0=xlo[:], in1=dinvb[:], op=mybir.AluOpType.mult
            )
            nc.vector.tensor_tensor(
                out=xhi[:], in0=xhi[:], in1=dinvb[:], op=mybir.AluOpType.mult
            )

            def Wquad(l, a, o):
                col = ((l * 2 + a) * 2 + o) * 16
                return W_sb[:, col : col + 16]

            def bvec(l, h):
                return b_sb[:, l * 2 + h : l * 2 + h + 1]

            def emit_gemm(kk, o, w):
                # u_kk = (dinv*x_kk) @ W_kk for columns [o, o+w) -> shard[kk]
                l = kk - 1
                uch = spool.tile([16, 2 * ECH], BF16, name=f"u{kk}_{o}", tag="chunk")
                for h in (0, 1):
                    psg = ppool.tile(
                        [16, ECH], F32, space="PSUM", name=f"g{kk}_{o}_{h}", tag="ps"
                    )
                    nc.tensor.matmul(
                        psg[:, :w], lhsT=Wquad(l, 0, h), rhs=xlo[:, o : o + w],
                        start=True, stop=False,
                    )
                    nc.tensor.matmul(
                        psg[:, :w], lhsT=Wquad(l, 1, h), rhs=xhi[:, o : o + w],
                        start=False, stop=True,
                    )
                    # cast + interleave write: u_chunk[p, 2j+h] = psg[p, j]
                    nc.vector.tensor_copy(
                        uch[:, h : 2 * w : 2].rearrange("p (w one) -> p w one", one=1),
                        psg[:, :w],
                    )
                if o < SPL:
                    nc.sync.dma_start(
                        shardA[kk][:, 2 * o : 2 * (o + w)], uch[:, : 2 * w]
                    )
                else:
                    nc.sync.dma_start(
                        shardB[kk][:, 2 * (o - SPL) : 2 * (o - SPL + w)],
                        uch[:, : 2 * w],
                    )
                if o + w == SPL:
                    nc.gpsimd.collective_compute(
                        "AllGather",
                        mybir.AluOpType.bypass,
                        replica_groups=rgroups,
                        ins=[shardA[kk].ap().opt()],
                        outs=[tabdA[kk].ap().opt()],
                    )
                elif o + w == SLOTS:
                    nc.gpsimd.collective_compute(
                        "AllGather",
                        mybir.AluOpType.bypass,
                        replica_groups=rgroups,
                        ins=[shardB[kk].ap().opt()],
                        outs=[tabdB[kk].ap().opt()],
                    )

            def emit_final(o, w):
                # logits + log_softmax for columns [o, o+w) -> out_dram
                psg = ppool.tile([16, ECH], F32, space="PSUM", name=f"gf_{o}", tag="ps")
                nc.tensor.matmul(
                    psg[:, :w], lhsT=Wquad(3, 0, 0), rhs=xlo[:, o : o + w],
                    start=True, stop=False,
                )
                nc.tensor.matmul(
                    psg[:, :w], lhsT=Wquad(3, 1, 0), rhs=xhi[:, o : o + w],
                    start=False, stop=True,
                )
                logits = spool.tile([16, ECH], F32, name=f"lg_{o}", tag="chunk")
                nc.vector.tensor_scalar_add(logits[:, :w], psg[:, :w], bvec(3, 0))
                ez = spool.tile([16, ECH], F32, name=f"e_{o}", tag="chunk")
                nc.scalar.activation(
                    out=ez[:, :w], in_=psg[:, :w],
                    func=mybir.ActivationFunctionType.Exp,
                    bias=bvec(3, 0), scale=1.0,
                )
                pss = ppool1.tile([1, ECH], F32, space="PSUM", name=f"sf_{o}", tag="ps1")
                nc.tensor.matmul(
                    pss[:, :w], lhsT=m6[:], rhs=ez[:, :w], start=True, stop=True
                )
                lnr = rpool.tile([1, ECH], F32, name=f"ln_{o}", tag="row")
                nc.scalar.activation(
                    out=lnr[:, :w], in_=pss[:, :w],
                    func=mybir.ActivationFunctionType.Ln,
                )
                psb = ppool.tile([16, ECH], F32, space="PSUM", name=f"bc_{o}", tag="ps")
                nc.tensor.matmul(
                    psb[:, :w], lhsT=o16[:], rhs=lnr[:, :w], start=True, stop=True
                )
                oc = spool.tile([16, ECH], F32, name=f"oc_{o}", tag="chunk")
                nc.vector.tensor_sub(oc[:, :w], logits[:, :w], psb[:, :w])
                nc.sync.dma_start(out_dram[:, o : o + w], oc[0:6, :w])

            # layer-1 GEMM from the (prescaled) input activations
            for o, w in echunks:
                emit_gemm(1, o, w)

            for k in (1, 2, 3):
                l = k - 1
                nc.sync.dma_start(table[:, : 2 * SPL], tabdA[k].ap())
                nc.sync.dma_start(table[:, 2 * SPL :], tabdB[k].ap())
                nc.vector.memset(partial[:], 0.0)

                # ---- gather + segment reduce ----
                for i in range(NCALLS):
                    it = ipool.tile([128, C // 16], I16, name=f"it{k}_{i}", tag="idx")
                    nc.sync.dma_start(
                        it[:], idx_in[:, i * (C // 16) : (i + 1) * (C // 16)]
                    )
                    msg = mpool.tile([128, 2 * C], BF16, name=f"m{k}_{i}", tag="msg")
                    nc.gpsimd.ap_gather(
                        out_ap=msg[:],
                        in_ap=table[:],
                        idxs_ap=it[:],
                        channels=128,
                        num_elems=SLOTS,
                        d=2,
                        num_idxs=C,
                    )
                    with nc.allow_low_precision(reason="bf16 partials"):
                        for q in range(NP):
                            if i >= len(pl.calls[q]):
                                continue
                            r0, n, P = pl.calls[q][i]
                            nc.vector.tensor_reduce(
                                out=partial[32 * q : 32 * q + 32, 2 * r0 : 2 * (r0 + n)],
                                in_=msg[32 * q : 32 * q + 32, : 2 * n * P].rearrange(
                                    "p (n P two) -> p n two P", P=P, two=2
                                ),
                                axis=mybir.AxisListType.X,
                                op=mybir.AluOpType.add,
                            )

                # ---- unpermute + self-term + chunk-sum + epilogue ----
                for uo, uw in uchunks:
                    iu = ipool.tile([128, UCH // 16], I16, name=f"iu{k}_{uo}", tag="idx")
                    nc.sync.dma_start(
                        iu[:, : uw // 16], up_in[:, uo // 16 : (uo + uw) // 16]
                    )
                    unp = mpool.tile([128, 2 * UCH], BF16, name=f"up{k}_{uo}", tag="msg")
                    nc.gpsimd.ap_gather(
                        out_ap=unp[:, : 2 * uw],
                        in_ap=partial[:],
                        idxs_ap=iu[:, : uw // 16],
                        channels=128,
                        num_elems=SLOTS,
                        d=2,
                        num_idxs=uw,
                    )
                    # unp += mask * u_own (self-loop term, natural order)
                    nc.vector.scalar_tensor_tensor(
                        out=unp[:, : 2 * uw],
                        in0=table[:, 2 * uo : 2 * (uo + uw)],
                        scalar=maskt[:],
                        in1=unp[:, : 2 * uw],
                        op0=mybir.AluOpType.mult,
                        op1=mybir.AluOpType.add,
                    )
                    # deinterleave halves
                    ua = upool.tile([128, UCH], BF16, name=f"ua{k}_{uo}", tag="und")
                    ub = upool.tile([128, UCH], BF16, name=f"ub{k}_{uo}", tag="und")
                    nc.vector.tensor_copy(
                        ua[:, :uw],
                        unp[:, : 2 * uw].rearrange("p (w two) -> p two w", two=2)[
                            :, 0:1, :
                        ],
                    )
                    nc.vector.tensor_copy(
                        ub[:, :uw],
                        unp[:, : 2 * uw].rearrange("p (w two) -> p two w", two=2)[
                            :, 1:2, :
                        ],
                    )
                    for so in range(0, uw, ECH):
                        w = min(ECH, uw - so)
                        o = uo + so
                        for h, uh in ((0, ua), (1, ub)):
                            pss = ppool.tile(
                                [16, ECH], F32, space="PSUM",
                                name=f"s{k}_{o}_{h}", tag="ps",
                            )
                            nc.tensor.matmul(
                                pss[:, :w], lhsT=s8[:], rhs=uh[:, so : so + w],
                                start=True, stop=True,
                            )
                            zc = spool.tile(
                                [16, ECH], BF16, name=f"z{k}_{o}_{h}", tag="chunk"
                            )
                            nc.vector.tensor_copy(zc[:, :w], pss[:, :w])
                            nc.vector.tensor_tensor(
                                out=zc[:, :w], in0=zc[:, :w],
                                in1=dinvb[:, o : o + w], op=mybir.AluOpType.mult,
                            )
                            xh = xlo if h == 0 else xhi
                            nc.scalar.activation(
                                out=xh[:, o : o + w], in_=zc[:, :w],
                                func=mybir.ActivationFunctionType.Relu,
                                bias=bvec(l, h), scale=1.0,
                            )
                            if k < 3:
                                nc.vector.tensor_tensor(
                                    out=xh[:, o : o + w], in0=xh[:, o : o + w],
                                    in1=dinvb[:, o : o + w], op=mybir.AluOpType.mult,
                                )
                        # fused next-stage for these freshly-written columns:
                        # hides under the remaining unperm gather calls
                        if k < 3:
                            if o + w > PER:
                                nc.vector.memset(xlo[:, PER:SLOTS], 0.0)
                                nc.vector.memset(xhi[:, PER:SLOTS], 0.0)
                            emit_gemm(k + 1, o, w)
                        else:
                            emit_final(o, w)

    nc.compile()
    return nc


# ---------------------------------------------------------------------------
# Entry point
# ---------------------------------------------------------------------------
def kernel(x, edge_index, W1, b1, W2, b2, W3, b3, Wf, bf):
    x = np.asarray(x, dtype=np.float32)
    pl = _build_plan(np.asarray(edge_index), x.shape[0])
    nc = _build_kernel(pl)
    in_maps = _make_in_maps(pl, x, W1, b1, W2, b2, W3, b3, Wf, bf)

    res = run_bass_kernel_spmd(nc, in_maps, core_ids=list(range(N_CORES)))

    LAST_RUN_INFO.clear()
    LAST_RUN_INFO["exec_time_ns"] = res.exec_time_ns
    LAST_RUN_INFO["mean_exec_time_ns"] = res.mean_exec_time_ns

    outs = [res.results[c]["out"] for c in range(N_CORES)]
    return _assemble_output(outs)
